# revision 11
# baseline (speedup 1.0000x reference)
"""Trainium2 Bass kernel for nn_AutoregressiveTokenHead (4-layer transformer
decoder with cross-attention + 32k-vocab head), SPMD over 8 NeuronCores.

Sharding: data-parallel over batch (2 batches/core); each core computes the
full 32k-vocab head for its own tokens.

Numerics: the large GEMMs (QK/V/CAQ/CAK/CAV/FFN1/head) run as fp8e4m3
DoubleRow matmuls with an exact hi+lo split of BOTH operands, computing
xh@wh + xh@wl + xl@wh (the dropped xl@wl term is ~2^-8). This is slightly
MORE accurate than bf16 while costing 0.75x the bf16 PE time under
DoubleRow (2 pair-products per instruction at 0.5 cycles/row). Attention
(scores/softmax/AV), the attention out-projections and FFN2 stay bf16.

Host side: token+position embedding (a gather), LayerNorm gain folding into
the adjacent matmul weights, fp8 hi/lo weight quantization with per-tensor
power-of-2 scales, dtype conversion. All bias inputs are zero for this
problem; a numpy fallback handles the (never exercised) nonzero-bias case.
"""
import sys
sys.path.insert(0, "/opt/trn_rl_repo")

import numpy as np
import ml_dtypes

import concourse.bass as bass
import concourse.bacc as bacc
import concourse.tile as tile
import concourse.mybir as mybir
from concourse.bass_utils import run_bass_kernel_spmd

# ---- problem constants (hardcoded per spec) ----
B, MEM, T = 16, 256, 128
D, H, L, V = 512, 8, 4, 32000
BOS = 32000
HD = D // H            # 64
DF = 4 * D             # 2048
NC = 8                 # cores
BL = B // NC           # 2 batches per core
TOK = BL * T           # 256 tokens per core
MTOK = BL * MEM        # 512 memory tokens per core
NEG = -1e9
EPS = 1e-5
KC = D // 128          # 4 contraction chunks of the model dim
NMB = MTOK // 128      # 4 memory-token tiles
NU = DF // 128         # 16 ffn-hidden chunks
SX = 4.0               # fp8 scale applied to LN outputs
SM = 4.0               # fp8 scale applied to memory features
ISQ = 0.125            # 1/sqrt(HD) softmax scale

F32 = mybir.dt.float32
BF16 = mybir.dt.bfloat16
FP8 = mybir.dt.float8e4
AF = mybir.ActivationFunctionType
OP = mybir.AluOpType
DR = mybir.MatmulPerfMode.DoubleRow

NP_FP8 = ml_dtypes.float8_e4m3
NP_BF16 = ml_dtypes.bfloat16

DEFAULT_CFG = dict(
    n_layers=L,         # layers to emit (debug)
    head=True,          # emit the vocab head (False -> output final x, debug)
)

# fp8-3term weight sites; scales are computed at prep time and passed into
# the build via cfg["scales"] (tuple of (name, float) pairs).
W8_SITES = ["qk", "v", "caq", "cak", "cav", "f1", "head"]


# ======================================================================
# device kernel builder
# ======================================================================

def build_kernel(cfg):
    sc = dict(cfg["scales"])
    nc = bacc.Bacc("TRN2", target_bir_lowering=False, debug=False, num_devices=NC)

    NLYR = cfg["n_layers"]

    # ---- DRAM parameters ----
    x0_d = nc.dram_tensor("x0", [TOK, D], F32, kind="ExternalInput")
    mem_d = nc.dram_tensor("mem", [MTOK, D], BF16, kind="ExternalInput")
    # fp8 hi/lo weights, layout [L, 2*D, n]: rows (two, k, p) with two=0 LO,
    # two=1 HI
    wqk_d = nc.dram_tensor("wqk8", [L, 2 * D, 2 * D], FP8, kind="ExternalInput")
    wv_d = nc.dram_tensor("wv8", [L, 2 * D, D], FP8, kind="ExternalInput")
    wcaq_d = nc.dram_tensor("wcaq8", [L, 2 * D, D], FP8, kind="ExternalInput")
    wcak_d = nc.dram_tensor("wcak8", [L, 2 * D, D], FP8, kind="ExternalInput")
    wcav_d = nc.dram_tensor("wcav8", [L, 2 * D, D], FP8, kind="ExternalInput")
    wf1_d = nc.dram_tensor("wf18", [L, 2 * D, DF], FP8, kind="ExternalInput")
    whead_d = nc.dram_tensor("whead8", [2 * D, V], FP8, kind="ExternalInput")
    # bf16 weights
    wsao_d = nc.dram_tensor("wsao", [L, D, D], BF16, kind="ExternalInput")
    wcao_d = nc.dram_tensor("wcao", [L, D, D], BF16, kind="ExternalInput")
    wf2_d = nc.dram_tensor("wf2", [L, DF, D], BF16, kind="ExternalInput")
    mask_d = nc.dram_tensor("mask01", [T, T], BF16, kind="ExternalInput")
    ident_d = nc.dram_tensor("ident", [128, 128], BF16, kind="ExternalInput")

    NVB = 32          # head column groups (1000 vocab each)
    WCW = V // NVB    # 1000
    VC = 250          # psum chunk width (DoubleRow moving limit 2*250<=512)
    NSUB = WCW // VC  # 4
    if cfg["head"]:
        out_d = nc.dram_tensor("out", [NVB, TOK, WCW], BF16, kind="ExternalOutput")
    else:
        out_d = nc.dram_tensor("out", [TOK, D], F32, kind="ExternalOutput")

    # descale constants
    c_qk = 1.0 / (SX * sc["qk"])
    c_v = 1.0 / (SX * sc["v"])
    c_caq = 1.0 / (SX * sc["caq"])
    c_cak = 1.0 / (SM * sc["cak"])
    c_cav = 1.0 / (SM * sc["cav"])
    c_f1 = 1.0 / (SX * sc["f1"])
    c_head = 1.0 / (SX * sc["head"])

    with tile.TileContext(nc) as tc:
        with (
            tc.tile_pool(name="const", bufs=1) as constp,
            tc.tile_pool(name="state", bufs=1) as statep,
            tc.tile_pool(name="work", bufs=2) as work,
            tc.tile_pool(name="soft", bufs=6) as soft,
            tc.tile_pool(name="wpre", bufs=1) as wpre,
            tc.tile_pool(name="ps_a", bufs=4, space="PSUM") as ps_a,
            tc.tile_pool(name="ps_tp", bufs=3, space="PSUM") as ps_tp,
            tc.tile_pool(name="ps_sc", bufs=2, space="PSUM") as ps_sc,
            tc.tile_pool(name="ps_att", bufs=2, space="PSUM") as ps_att,
        ):
            # ---------- constants ----------
            ident = constp.tile([128, 128], BF16, name="ident", tag="ident")
            nc.sync.dma_start(ident[:], ident_d[:])
            mask01 = constp.tile([T, T], BF16, name="mask01", tag="mask01")
            nc.sync.dma_start(mask01[:], mask_d[:])

            # ---------- load x0 (one DMA; x[b] are slices) ----------
            x_all = statep.tile([128, BL * D], F32, name="x_all", tag="x_all")
            nc.sync.dma_start(
                x_all[:].rearrange("p (b d) -> p b d", b=BL),
                x0_d[:].rearrange("(b p) d -> p b d", p=T))
            x = [x_all[:, b * D:(b + 1) * D] for b in range(BL)]
            sx0 = soft.tile([128, BL], F32, name="sx0", tag="sx", bufs=3)
            for b in range(BL):
                scr0 = work.tile([T, D], BF16, name="ln_scr", tag="ln_scr")
                nc.scalar.activation(
                    scr0[:], x[b], AF.Identity, accum_out=sx0[:, b:b + 1])

            # ---------- memory -> memT8 (feature-major, fp8 hi/lo) ----------
            m_sb = work.tile([128, NMB * D], BF16, name="m_sb", tag="m_sb", bufs=1)
            nc.sync.dma_start(
                m_sb[:].rearrange("p (mb d) -> p mb d", mb=NMB),
                mem_d[:].rearrange("(mb p) d -> p mb d", p=128))
            memT8 = statep.tile([128, 2, KC, MTOK], FP8, name="memT8", tag="memT8")
            for mb in range(NMB):
                tp4 = ps_tp.tile([128, KC, 128], BF16, name="tpm", tag="tp")
                for k in range(KC):
                    nc.tensor.transpose(
                        tp4[:, k, :],
                        m_sb[:, mb * D + k * 128:mb * D + (k + 1) * 128],
                        ident[:])
                dst_hi = memT8[:, 0, :, mb * 128:(mb + 1) * 128]
                dst_lo = memT8[:, 1, :, mb * 128:(mb + 1) * 128]
                nc.scalar.activation(dst_hi, tp4[:], AF.Copy, scale=SM)
                nc.vector.scalar_tensor_tensor(
                    dst_lo, tp4[:], SM, dst_hi, OP.mult, OP.subtract)

            # rsqrt magic constant for Newton iterations (no ACT table needed)
            magic = constp.tile([128, 4], mybir.dt.int32, name="magic", tag="magic")
            nc.vector.memset(magic[:], 0x5f3759df)

            # ---------- helpers ----------
            def layer_norm_to(xt_list, sx, hT8):
                """LN (gain/bias folded into next matmul) -> fp8 hi/lo tiles.

                Writes hT8 [128, 2, KC, TOK] (feature-major, scaled by SX via
                the copies). The small-vector chain runs once per LN on
                [128, BL] columns (both batches at once). rstd via
                inverse-sqrt bit trick + 2 Newton steps; eps dropped (5e-6
                relative, far below the fp8 noise floor).
                """
                ssq = soft.tile([128, BL], F32, name="ln_ssq", tag="ln_ssq")
                for b in range(BL):
                    scr = work.tile([T, D], BF16, name="ln_scr", tag="ln_scr")
                    nc.scalar.activation(
                        scr[:], xt_list[b], AF.Square, accum_out=ssq[:, b:b + 1])
                mean = soft.tile([128, BL], F32, name="ln_mean", tag="ln_mean")
                nc.vector.tensor_scalar_mul(mean[:], sx[:], 1.0 / D)
                m2 = soft.tile([128, BL], F32, name="ln_m2", tag="ln_m2")
                nc.vector.tensor_tensor(m2[:], mean[:], mean[:], OP.mult)
                w = soft.tile([128, BL], F32, name="ln_w", tag="ln_w")
                nc.vector.scalar_tensor_tensor(
                    w[:], ssq[:], 1.0 / D, m2[:], OP.mult, OP.subtract)
                yb = soft.tile([128, BL], mybir.dt.int32, name="ln_yb", tag="ln_yb")
                nc.vector.tensor_scalar(
                    yb[:], w[:].bitcast(mybir.dt.int32), 1, None,
                    OP.arith_shift_right)
                y = soft.tile([128, BL], F32, name="ln_y", tag="ln_y")
                nc.vector.tensor_tensor(
                    y[:].bitcast(mybir.dt.int32), magic[:, :BL], yb[:], OP.subtract)
                t1 = soft.tile([128, BL], F32, name="ln_t1", tag="ln_t1")
                for _ in range(2):  # Newton: y *= 1.5 - 0.5*w*y^2
                    nc.vector.tensor_tensor(t1[:], y[:], y[:], OP.mult)
                    nc.vector.tensor_tensor(t1[:], t1[:], w[:], OP.mult)
                    nc.vector.tensor_scalar(t1[:], t1[:], -0.5, 1.5, OP.mult, OP.add)
                    nc.vector.tensor_tensor(y[:], y[:], t1[:], OP.mult)
                mrstd = soft.tile([128, BL], F32, name="ln_nmr", tag="ln_nmr")
                nc.vector.tensor_tensor(mrstd[:], mean[:], y[:], OP.mult)
                for b in range(BL):
                    hh = work.tile([T, D], BF16, name=f"hh{b}", tag=f"hh{b}")
                    nc.vector.tensor_scalar(
                        hh[:], xt_list[b], y[:, b:b + 1], mrstd[:, b:b + 1],
                        OP.mult, OP.subtract)
                    # transpose to feature-major and split hi/lo fp8 (x SX)
                    tp4 = ps_tp.tile([128, KC, T], BF16, name="tp4", tag="tp")
                    for kk in range(KC):
                        nc.tensor.transpose(
                            tp4[:, kk, :], hh[:, kk * 128:(kk + 1) * 128], ident[:])
                    dst_hi = hT8[:, 0, :, b * T:(b + 1) * T]
                    dst_lo = hT8[:, 1, :, b * T:(b + 1) * T]
                    nc.scalar.activation(dst_hi, tp4[:], AF.Copy, scale=SX)
                    nc.vector.scalar_tensor_tensor(
                        dst_lo, tp4[:], SX, dst_hi, OP.mult, OP.subtract)

            def residual(ps, b, sx, scalar=1.0):
                """x[b] += psum*scalar, accumulating row sums into sx[:, b]."""
                nc.vector.scalar_tensor_tensor(
                    x[b], ps, scalar, x[b], OP.mult, OP.add,
                    accum_out=sx[:, b:b + 1])

            def load_w8(w_dram, l, n_feat, tag, bufs=1):
                """[L, 2D, n] fp8 hi/lo weight -> tile [128, 2, KC, n]."""
                wt = wpre.tile([128, 2, KC, n_feat], FP8, name=tag, tag=tag,
                               bufs=bufs)
                nc.sync.dma_start(
                    wt[:],
                    w_dram[l].rearrange("(two k p) n -> p two k n", p=128, two=2))
                return wt

            def load_w16(w_dram, l, kdim, n_feat, tag, bufs=1):
                """[L, kdim*128, n] bf16 weight -> tile [128, kdim, n]."""
                wt = wpre.tile([128, kdim, n_feat], BF16, name=tag, tag=tag,
                               bufs=bufs)
                nc.sync.dma_start(
                    wt[:],
                    w_dram[l].rearrange("(k p) n -> p k n", p=128))
                return wt

            def mm3_hh(ps_sl, lhs8, rhs8, m_off, m_sz, n_off, n_sz, first=True):
                """hi@hi terms: 2 DoubleRow insts over k-chunk pairs.
                lhs8/rhs8 are [128, 2, KC, n] hi/lo tiles; lhsT comes from
                lhs8's hi plane, rhs from rhs8's hi plane."""
                lhs_hi = 1 if lhs8.shape[1] == 2 and lhs8 is not None else 1
                for kp in range(KC // 2):
                    nc.tensor.matmul(
                        ps_sl,
                        lhs8[:, lhs8_hi_plane(lhs8), 2 * kp:2 * kp + 2,
                             m_off:m_off + m_sz],
                        rhs8[:, rhs8_hi_plane(rhs8), 2 * kp:2 * kp + 2,
                             n_off:n_off + n_sz],
                        start=(first and kp == 0), stop=False, perf_mode=DR)

            def mm3_cross(ps_sl, lhs8, rhs8, m_off, m_sz, n_off, n_sz,
                          last=True):
                """Cross terms hi@lo + lo@hi: 4 DR insts (one per k-chunk),
                pairing lhs plane order against the opposite rhs plane
                order (lhs (a,b) x rhs (b',a'))."""
                for k in range(KC):
                    nc.tensor.matmul(
                        ps_sl,
                        lhs8[:, :, k, m_off:m_off + m_sz],
                        rhs8[:, :, k, n_off:n_off + n_sz],
                        start=False, stop=(last and k == KC - 1), perf_mode=DR)

            # ================= transformer layers =================
            sx_cur = sx0
            for l in range(NLYR):
                # ---------- CA K/V projections (independent of x; emitted
                # early so the scheduler can fill PE gaps) ----------
                wcak_t = load_w8(wcak_d, l, D, "wcak_t")
                kcaT = work.tile([128, KC, MTOK], BF16, name="kcaT", tag="kcaT",
                                 bufs=2)
                for m in range(KC):
                    ps = ps_a.tile([128, MTOK], F32, name="mm", tag="mm")
                    for h in range(2):
                        mm3(ps[:, h * 256:(h + 1) * 256], wcak_t, memT8,
                            m * 128, 128, h * 256, 256, True, True)
                    nc.scalar.activation(kcaT[:, m, :], ps[:], AF.Copy,
                                         scale=c_cak)

                wcav_t = load_w8(wcav_d, l, D, "wcav_t")
                vca = work.tile([128, NMB * D], BF16, name="vca", tag="vca",
                                bufs=2)
                for mb in range(NMB):
                    ps = ps_a.tile([128, D], F32, name="mm", tag="mm")
                    for h in range(2):
                        mm3_xw(ps[:, h * 256:(h + 1) * 256], memT8, wcav_t,
                               mb * 128, 128, h * 256, 256, True, True)
                    nc.vector.tensor_scalar_mul(
                        vca[:, mb * D:(mb + 1) * D], ps[:], c_cav)

                # ---------- self-attention ----------
                hT8 = work.tile([128, 2, KC, TOK], FP8, name="hT8a", tag="hT8a")
                layer_norm_to(x, sx_cur, hT8)

                wqk_t = load_w8(wqk_d, l, 2 * D, "wqk_t")
                qkT = work.tile([128, 2 * KC, TOK], BF16, name="qkT", tag="qkT")
                for m in range(2 * KC):
                    ps = ps_a.tile([128, TOK], F32, name="mm", tag="mm")
                    mm3(ps[:], wqk_t, hT8, m * 128, 128, 0, TOK, True, True)
                    nc.scalar.activation(
                        qkT[:, m, :], ps[:], AF.Copy,
                        scale=(c_qk * ISQ) if m < KC else c_qk)

                # v token-major: v_sb[b] [T, D] slices
                wv_t = load_w8(wv_d, l, D, "wv_t")
                v_sb = work.tile([T, BL * D], BF16, name="v_sb", tag="v_sb",
                                 bufs=1)
                for b in range(BL):
                    ps = ps_a.tile([T, D], F32, name="mm", tag="mm")
                    for h in range(2):
                        mm3_xw(ps[:, h * 256:(h + 1) * 256], hT8, wv_t,
                               b * T, T, h * 256, 256, True, True)
                    nc.vector.tensor_scalar_mul(
                        v_sb[:, b * D:(b + 1) * D], ps[:], c_v)

                # attention per (b, h)
                attnT = work.tile([128, KC, TOK], BF16, name="attnT",
                                  tag="attnT", bufs=2)
                for b in range(BL):
                    for hp in range(H // 2):      # head pairs (h=2hp, 2hp+1)
                        mq = hp
                        tp2 = ps_tp.tile([T, 2 * T], BF16, name="tp", tag="tp")
                        for hi in range(2):
                            po = hi * HD
                            q_sl = qkT[po:po + HD, mq, b * T:(b + 1) * T]
                            k_sl = qkT[po:po + HD, KC + mq, b * T:(b + 1) * T]
                            s_ps = ps_a.tile([T, T], F32, name="mm", tag="mm")
                            nc.tensor.matmul(s_ps[:], q_sl, k_sl, start=True,
                                             stop=True)
                            p_raw = soft.tile([T, T], BF16, name="p_raw",
                                              tag="p_raw")
                            nc.scalar.activation(p_raw[:], s_ps[:], AF.Exp)
                            pm = soft.tile([T, T], BF16, name="pm", tag="pm")
                            r = soft.tile([T, 1], F32, name="r", tag="r")
                            nc.vector.scalar_tensor_tensor(
                                pm[:], p_raw[:], 1.0, mask01[:], OP.mult,
                                OP.mult, accum_out=r[:])
                            rinv = soft.tile([T, 1], F32, name="rinv", tag="rinv")
                            nc.vector.reciprocal(rinv[:], r[:])
                            pn = soft.tile([T, T], BF16, name="pn", tag="pn")
                            nc.vector.tensor_scalar_mul(pn[:], pm[:], rinv[:])
                            nc.tensor.transpose(
                                tp2[:, hi * T:(hi + 1) * T], pn[:], ident[:])
                        pT = soft.tile([T, 2 * T], BF16, name="pT", tag="pT")
                        nc.vector.tensor_copy(pT[:], tp2[:])
                        a_ps = ps_att.tile([128, T], F32, name="att", tag="att")
                        for hi in range(2):
                            h = 2 * hp + hi
                            nc.tensor.matmul(
                                a_ps[hi * HD:(hi + 1) * HD, :],
                                v_sb[:, b * D + h * HD:b * D + (h + 1) * HD],
                                pT[:, hi * T:(hi + 1) * T], start=True, stop=True)
                        nc.vector.tensor_copy(
                            attnT[:, mq, b * T:(b + 1) * T], a_ps[:])

                # out projection + residual (bf16)
                wsao_t = load_w16(wsao_d, l, KC, D, "wsao_t")
                sx_cur = soft.tile([128, BL], F32, name="sx_sa", tag="sx", bufs=3)
                for b in range(BL):
                    y_ps = ps_a.tile([T, D], F32, name="mm", tag="mm")
                    for k in range(KC):
                        nc.tensor.matmul(
                            y_ps[:], attnT[:, k, b * T:(b + 1) * T],
                            wsao_t[:, k, :],
                            start=(k == 0), stop=(k == KC - 1))
                    residual(y_ps[:], b, sx_cur)

                # ---------- cross-attention ----------
                hT8b = work.tile([128, 2, KC, TOK], FP8, name="hT8b", tag="hT8b")
                layer_norm_to(x, sx_cur, hT8b)

                wcaq_t = load_w8(wcaq_d, l, D, "wcaq_t")
                qcaT = work.tile([128, KC, TOK], BF16, name="qcaT", tag="qcaT")
                for m in range(KC):
                    ps = ps_a.tile([128, TOK], F32, name="mm", tag="mm")
                    mm3(ps[:], wcaq_t, hT8b, m * 128, 128, 0, TOK, True, True)
                    nc.scalar.activation(qcaT[:, m, :], ps[:], AF.Copy,
                                         scale=c_caq * ISQ)

                attnC = work.tile([128, KC, TOK], BF16, name="attnC",
                                  tag="attnC", bufs=1)
                for b in range(BL):
                    for hp in range(H // 2):
                        mq = hp
                        a_ps = ps_att.tile([128, T], F32, name="att", tag="att")
                        for hi in range(2):
                            h = 2 * hp + hi
                            po = hi * HD
                            q_sl = qcaT[po:po + HD, mq, b * T:(b + 1) * T]
                            k_sl = kcaT[po:po + HD, mq, b * MEM:(b + 1) * MEM]
                            s_ps = ps_a.tile([T, MEM], F32, name="mm", tag="mm")
                            nc.tensor.matmul(s_ps[:], q_sl, k_sl, start=True,
                                             stop=True)
                            p = soft.tile([T, MEM], BF16, name="pc", tag="pc")
                            r = soft.tile([T, 1], F32, name="r", tag="r")
                            nc.scalar.activation(p[:], s_ps[:], AF.Exp,
                                                 accum_out=r[:])
                            rinv = soft.tile([T, 1], F32, name="rinv", tag="rinv")
                            nc.vector.reciprocal(rinv[:], r[:])
                            pn = soft.tile([T, MEM], BF16, name="pnc", tag="pnc")
                            nc.vector.tensor_scalar_mul(pn[:], p[:], rinv[:])
                            tp2 = ps_tp.tile([128, 2 * T], BF16, name="tp",
                                             tag="tp")
                            for j in range(MEM // 128):
                                nc.tensor.transpose(
                                    tp2[:, j * T:(j + 1) * T],
                                    pn[:, j * 128:(j + 1) * 128], ident[:])
                            pT = soft.tile([128, 2 * T], BF16, name="pT",
                                           tag="pT")
                            nc.vector.tensor_copy(pT[:], tp2[:])
                            for j in range(MEM // 128):
                                mbi = b * (MEM // 128) + j
                                nc.tensor.matmul(
                                    a_ps[po:po + HD, :],
                                    vca[:, mbi * D + h * HD:mbi * D + (h + 1) * HD],
                                    pT[:, j * T:(j + 1) * T],
                                    start=(j == 0), stop=(j == MEM // 128 - 1))
                        nc.vector.tensor_copy(
                            attnC[:, mq, b * T:(b + 1) * T], a_ps[:])

                wcao_t = load_w16(wcao_d, l, KC, D, "wcao_t")
                sx_cur = soft.tile([128, BL], F32, name="sx_ca", tag="sx", bufs=3)
                for b in range(BL):
                    yc_ps = ps_a.tile([T, D], F32, name="mm", tag="mm")
                    for k in range(KC):
                        nc.tensor.matmul(
                            yc_ps[:], attnC[:, k, b * T:(b + 1) * T],
                            wcao_t[:, k, :],
                            start=(k == 0), stop=(k == KC - 1))
                    residual(yc_ps[:], b, sx_cur)

                # ---------- FFN ----------
                hT8c = work.tile([128, 2, KC, TOK], FP8, name="hT8c", tag="hT8c")
                layer_norm_to(x, sx_cur, hT8c)

                wf1_t = load_w8(wf1_d, l, DF, "wf1_t")
                uT = work.tile([128, NU, TOK], BF16, name="uT", tag="uT", bufs=1)
                for m in range(NU):
                    u_ps = ps_a.tile([128, TOK], F32, name="mm", tag="mm")
                    mm3(u_ps[:], wf1_t, hT8c, m * 128, 128, 0, TOK, True, True)
                    nc.scalar.activation(uT[:, m, :], u_ps[:], AF.Gelu,
                                         scale=c_f1)

                wf2_t = load_w16(wf2_d, l, NU, D, "wf2_t")
                sx_cur = soft.tile([128, BL], F32, name="sx_f", tag="sx", bufs=3)
                for b in range(BL):
                    yf_ps = ps_a.tile([T, D], F32, name="mm", tag="mm")
                    for m in range(NU):
                        nc.tensor.matmul(
                            yf_ps[:], uT[:, m, b * T:(b + 1) * T],
                            wf2_t[:, m, :],
                            start=(m == 0), stop=(m == NU - 1))
                    residual(yf_ps[:], b, sx_cur)

            # ================= final LN + head =================
            if not cfg["head"]:
                for b in range(BL):
                    nc.sync.dma_start(out_d[b * T:(b + 1) * T, :], x[b])
            else:
                xfT8 = statep.tile([128, 2, KC, TOK], FP8, name="xfT8",
                                   tag="xfT8")
                layer_norm_to(x, sx_cur, xfT8)
                # head: stream fp8 hi/lo weight chunks; 3-term DR matmuls;
                # psum -> staging copies alternate Act/DVE; DMA out.
                for vb in range(NVB):
                    wh = wpre.tile([128, 2, KC, WCW], FP8, name="wh_t",
                                   tag="wh_t", bufs=5)
                    vo = vb * WCW
                    nc.sync.dma_start(
                        wh[:],
                        whead_d[:, vo:vo + WCW]
                        .rearrange("(two k p) v -> p two k v", p=128, two=2))
                    for tt in range(TOK // T):
                        stg = work.tile([T, WCW], BF16, name="ostage",
                                        tag="ostage", bufs=4)
                        o_ps = [ps_a.tile([T, VC], F32, name="mm", tag="mm")
                                for _ in range(NSUB)]
                        for sub in range(NSUB):
                            mm3_xw(o_ps[sub][:], xfT8, wh, tt * T, T,
                                   sub * VC, VC, True, True)
                        for sub in range(NSUB):
                            dst = stg[:, sub * VC:(sub + 1) * VC]
                            if sub % 2 == 0:
                                nc.scalar.activation(dst, o_ps[sub][:],
                                                     AF.Copy, scale=c_head)
                            else:
                                nc.vector.tensor_scalar_mul(
                                    dst, o_ps[sub][:], c_head)
                        nc.sync.dma_start(
                            out_d[vb, tt * T:(tt + 1) * T, :], stg[:])

    nc.compile()
    return nc


# ======================================================================
# host side
# ======================================================================

def _q8_split(w):
    """fp8 hi/lo split with a power-of-2 per-tensor scale. Returns
    (packed [2, KC, 128, n] fp8 array with [0]=LO [1]=HI, scale)."""
    w = np.asarray(w, np.float32)
    rms = float(np.sqrt(np.mean(w.astype(np.float64) ** 2)))
    s = float(2.0 ** np.round(np.log2(4.0 / max(rms, 1e-30))))
    ws = w * s
    hi = ws.astype(NP_FP8)
    lo = (ws - hi.astype(np.float32)).astype(NP_FP8)
    kdim, n = w.shape
    packed = np.empty((2, KC, 128, n), NP_FP8)
    packed[0] = lo.reshape(KC, 128, n)
    packed[1] = hi.reshape(KC, 128, n)
    return packed, s


def _prep_inputs(inputs):
    """Fold params, embed tokens, quantize weights, build the 8 per-core
    input maps. Returns (in_maps, scales)."""
    f32 = np.float32
    tok_emb = np.asarray(inputs["tok_emb"], f32)
    pos_emb = np.asarray(inputs["pos_emb"], f32)
    targets = np.asarray(inputs["targets"])
    memory = np.asarray(inputs["memory"], f32)

    inp = np.concatenate(
        [np.full((B, 1), BOS, dtype=targets.dtype), targets[:, :-1]], axis=1)
    x0 = tok_emb[inp] + pos_emb[:T][None]          # [B, T, D] f32
    x0 = np.ascontiguousarray(x0, f32)

    def fold(w, g):
        return np.asarray(g, f32)[:, None] * np.asarray(w, f32)

    wsao = np.empty((L, D, D), NP_BF16)
    wcao = np.empty((L, D, D), NP_BF16)
    wf2 = np.empty((L, DF, D), NP_BF16)
    scales = {}

    # per-site scale is shared across layers (single descale constant);
    # compute it from the full stacked tensor.
    def q8_all(ws):  # ws [L, kdim, n]
        ws = np.asarray(ws, f32)
        rms = float(np.sqrt(np.mean(ws.astype(np.float64) ** 2)))
        s = float(2.0 ** np.round(np.log2(4.0 / max(rms, 1e-30))))
        out = np.empty((ws.shape[0], 2, KC, 128, ws.shape[2]), NP_FP8)
        for l in range(ws.shape[0]):
            wsl = ws[l] * s
            hi = wsl.astype(NP_FP8)
            lo = (wsl - hi.astype(np.float32)).astype(NP_FP8)
            out[l, 0] = lo.reshape(KC, 128, -1)
            out[l, 1] = hi.reshape(KC, 128, -1)
        return out, s

    qk_f = np.empty((L, D, 2 * D), f32)
    v_f = np.empty((L, D, D), f32)
    caq_f = np.empty((L, D, D), f32)
    cak_f = np.empty((L, D, D), f32)
    cav_f = np.empty((L, D, D), f32)
    f1_f = np.empty((L, D, DF), f32)
    for l in range(L):
        wqkv = fold(inputs["sa_qkv_w"][l], inputs["ln1_g"][l])
        qk_f[l] = wqkv[:, :2 * D]
        v_f[l] = wqkv[:, 2 * D:]
        wsao[l] = np.asarray(inputs["sa_out_w"][l], f32).astype(NP_BF16)
        caq_f[l] = fold(inputs["ca_q_w"][l], inputs["ln2_g"][l])
        ckv = np.asarray(inputs["ca_kv_w"][l], f32)
        cak_f[l] = ckv[:, :D]
        cav_f[l] = ckv[:, D:]
        wcao[l] = np.asarray(inputs["ca_out_w"][l], f32).astype(NP_BF16)
        f1_f[l] = fold(inputs["ffn1_w"][l], inputs["ln3_g"][l])
        wf2[l] = np.asarray(inputs["ffn2_w"][l], f32).astype(NP_BF16)

    wqk8, scales["qk"] = q8_all(qk_f)
    wv8, scales["v"] = q8_all(v_f)
    wcaq8, scales["caq"] = q8_all(caq_f)
    wcak8, scales["cak"] = q8_all(cak_f)
    wcav8, scales["cav"] = q8_all(cav_f)
    wf18, scales["f1"] = q8_all(f1_f)

    whead_f = np.asarray(inputs["normf_g"], f32)[:, None] * np.asarray(
        inputs["out_w"], f32)
    whead8_p, s_head = _q8_split(whead_f)
    scales["head"] = s_head

    mask01 = np.tril(np.ones((T, T), f32)).astype(NP_BF16)
    ident = np.eye(128, dtype=f32).astype(NP_BF16)

    shared = {
        "wqk8": wqk8.reshape(L, 2 * D, 2 * D),
        "wv8": wv8.reshape(L, 2 * D, D),
        "wcaq8": wcaq8.reshape(L, 2 * D, D),
        "wcak8": wcak8.reshape(L, 2 * D, D),
        "wcav8": wcav8.reshape(L, 2 * D, D),
        "wf18": wf18.reshape(L, 2 * D, DF),
        "whead8": whead8_p.reshape(2 * D, V),
        "wsao": wsao, "wcao": wcao, "wf2": wf2,
        "mask01": mask01, "ident": ident,
    }
    in_maps = []
    for c in range(NC):
        m = dict(shared)
        m["x0"] = np.ascontiguousarray(
            x0[c * BL:(c + 1) * BL].reshape(TOK, D))
        m["mem"] = np.ascontiguousarray(
            memory[c * BL:(c + 1) * BL].reshape(MTOK, D)).astype(NP_BF16)
        in_maps.append(m)
    return in_maps, scales


def _biases_trivial(inputs):
    for k in ("sa_qkv_b", "sa_out_b", "ca_q_b", "ca_kv_b", "ca_out_b",
              "ffn1_b", "ffn2_b", "ln1_b", "ln2_b", "ln3_b", "normf_b"):
        if np.any(np.asarray(inputs[k])):
            return False
    return True


def _numpy_fallback(inputs):
    """Exact (slow) host fallback, used only if bias inputs are nonzero."""
    try:
        from scipy.special import erf
    except ImportError:
        import math
        erf = np.vectorize(math.erf)

    f = {k: (np.asarray(v) if np.issubdtype(np.asarray(v).dtype, np.integer)
             else np.asarray(v, np.float32)) for k, v in inputs.items()}

    def ln(x, g, b):
        m = x.mean(-1, keepdims=True)
        v = ((x - m) ** 2).mean(-1, keepdims=True)
        return (x - m) / np.sqrt(v + EPS) * g + b

    def split(t):
        return t.reshape(t.shape[0], t.shape[1], H, HD).transpose(0, 2, 1, 3)

    def merge(t):
        return t.transpose(0, 2, 1, 3).reshape(t.shape[0], t.shape[2], D)

    def softmax(s):
        s = s - s.max(-1, keepdims=True)
        e = np.exp(s)
        return e / e.sum(-1, keepdims=True)

    targets = f["targets"]
    inp = np.concatenate(
        [np.full((B, 1), BOS, dtype=targets.dtype), targets[:, :-1]], axis=1)
    x = f["tok_emb"][inp] + f["pos_emb"][:T][None]
    causal = np.tril(np.ones((T, T), bool))
    scale = 1.0 / np.sqrt(HD)
    for l in range(L):
        h = ln(x, f["ln1_g"][l], f["ln1_b"][l])
        qkv = h @ f["sa_qkv_w"][l] + f["sa_qkv_b"][l]
        q, k, v = np.split(qkv, 3, axis=-1)
        q, k, v = split(q), split(k), split(v)
        s = np.einsum('bhqd,bhkd->bhqk', q, k) * scale
        a = softmax(np.where(causal, s, NEG))
        x = x + merge(np.einsum('bhqk,bhkd->bhqd', a, v)) @ f["sa_out_w"][l] + f["sa_out_b"][l]
        h = ln(x, f["ln2_g"][l], f["ln2_b"][l])
        q = split(h @ f["ca_q_w"][l] + f["ca_q_b"][l])
        kv = f["memory"] @ f["ca_kv_w"][l] + f["ca_kv_b"][l]
        k, v = np.split(kv, 2, axis=-1)
        k, v = split(k), split(v)
        s = np.einsum('bhqd,bhkd->bhqk', q, k) * scale
        a = softmax(s)
        x = x + merge(np.einsum('bhqk,bhkd->bhqd', a, v)) @ f["ca_out_w"][l] + f["ca_out_b"][l]
        h = ln(x, f["ln3_g"][l], f["ln3_b"][l])
        g = h @ f["ffn1_w"][l] + f["ffn1_b"][l]
        g = 0.5 * g * (1 + erf(g / np.sqrt(2.0)))
        x = x + g @ f["ffn2_w"][l] + f["ffn2_b"][l]
    x = ln(x, f["normf_g"], f["normf_b"])
    return (x @ f["out_w"] + f["out_b"]).astype(np.float32)


_BUILT = {}


def get_built(cfg=None):
    cfg = dict(DEFAULT_CFG, **(cfg or {}))
    if "scales" not in cfg:
        # default scales for standalone builds (test.py timing path); the
        # real run always passes prep-computed scales.
        cfg["scales"] = tuple(sorted(
            {s: 256.0 for s in W8_SITES}.items()))
    cfg_key = tuple(sorted((k, v) for k, v in cfg.items()))
    if cfg_key not in _BUILT:
        _BUILT[cfg_key] = build_kernel(cfg)
    return _BUILT[cfg_key], cfg


def run_device(inputs, cfg=None):
    in_maps, scales = _prep_inputs(inputs)
    cfg = dict(DEFAULT_CFG, **(cfg or {}))
    cfg["scales"] = tuple(sorted(scales.items()))
    nc, cfg = get_built(cfg)
    res = run_bass_kernel_spmd(nc, in_maps, core_ids=list(range(NC)))
    outs = [res.results[c]["out"] for c in range(NC)]
    if not cfg["head"]:
        # debug: final residual stream per core -> [B, T, D]
        return np.concatenate([o.reshape(BL, T, D) for o in outs], axis=0)
    # per-core out: [32 vocab-groups, 256 tok, 1000] for that core's batches
    logits = np.empty((B, T, V), np.float32)
    for c in range(NC):
        oc = outs[c].transpose(1, 0, 2).reshape(BL, T, V).astype(np.float32)
        logits[c * BL:(c + 1) * BL] = oc
    out_b = np.asarray(inputs["out_b"], np.float32)
    normf_b = np.asarray(inputs["normf_b"], np.float32)
    bias = normf_b @ np.asarray(inputs["out_w"], np.float32) + out_b
    if np.any(bias):
        logits = logits + bias
    return logits


def kernel(**inputs) -> np.ndarray:
    if not _biases_trivial(inputs):
        return _numpy_fallback(inputs)
    return run_device(inputs)
